# revision 17
# baseline (speedup 1.0000x reference)
"""Trainium2 Bass kernel for nn_Ensemble_of_ensemble (topk_masking).

Pure data-parallel over batch: 8192 rows split across 8 NeuronCores
(1024 rows each). Per core:
  - x viewed as [16384 (row,model) pairs, 345 classes], streamed as two
    [128, 32*345] "pair tiles" per super-group
  - per-pair-row softmax sum (se) via big ACT exp + DVE tensor_scalar accum
  - label logits gathered with GPSIMD ap_gather (per-16-partition groups
    share a row -> per-group index lists)
  - per-row model softmax / top-4 threshold (DVE max8) / L1 norms in a
    transposed [64 B-tiles, 128 (row_off, model)] smalls layout
  - both ensemble sums via fp32 TensorE matmuls with block-diagonal
    weights; w/p interleaved 16+16 per B-tile, packed 8 B-tiles per PSUM
    bank using 32-strip col tiling
  - ensemble CE via ACT exp+accum directly from PSUM + ap_gather
Scalar losses are finalized on host from per-row partial sums.
"""
import os
import numpy as np
from contextlib import ExitStack

import concourse.bass as bass
import concourse.bacc as bacc
import concourse.mybir as mybir
from concourse.tile import TileContext
from concourse import library_config

F32 = mybir.dt.float32
F32R = mybir.dt.float32r
I16 = mybir.dt.int16
ALU = mybir.AluOpType
ACTF = mybir.ActivationFunctionType
AX = mybir.AxisListType

B, M, C, K = 8192, 16, 345, 4
NCORES = 8
BL = B // NCORES            # 1024 rows per core
PAIRS = BL * M              # 16384 (row, model) pairs per core
NPAIR = 4                   # pair-tiles per core ([128, 32*345])
NSG = 4                     # super groups (one pair-tile each)
PPS = NPAIR // NSG          # pair-tiles per super group (1)
JL = 32                     # B-tiles per super group
NH = 16                     # half-A-tiles per core (psum groups of 8 B-tiles)
CP = C + 1                  # padded class stride (346, even)

USE_F32R = bool(int(os.environ.get("KERNEL_F32R", "0")))


def _patch_act_tables():
    # Exp and Ln interleave heavily; restrict the table-set choices to ones
    # containing BOTH so insert_act_table_loads emits a single load instead
    # of thrashing between exp_and_others and natural_log (~2.7us each).
    import concourse.hw_specs as hw_specs
    if getattr(bacc, "_act_tables_patched", False):
        return
    orig = bacc.get_activation_tables

    def filtered(arch):
        # Keep ALL sets in original order (act_func_set_id is the index
        # into act_info.json) but hide Exp/Ln from every set that doesn't
        # contain both, so the load-insertion pass always picks the
        # combined set for them.
        tabs = orig(arch)
        need = {ACTF.Exp, ACTF.Ln}
        if not any(need <= v for v in tabs.values()):
            return tabs
        out = {}
        for k, v in tabs.items():
            out[k] = set(v) if need <= v else set(v) - need
        return out

    bacc.get_activation_tables = filtered
    bacc._act_tables_patched = True


def build_nc():
    _patch_act_tables()
    nc = bacc.Bacc("TRN2", target_bir_lowering=False, debug=False,
                   num_devices=NCORES)

    xdt = F32R if USE_F32R else F32
    d_x = nc.dram_tensor("x", [PAIRS, C], xdt, kind="ExternalInput")
    d_gidx = nc.dram_tensor("gidx", [NPAIR, 128, 2], I16, kind="ExternalInput")
    d_lidx = nc.dram_tensor("lidx", [NH, 128], I16, kind="ExternalInput")
    d_pat8 = nc.dram_tensor("pat8", [128, 8], F32, kind="ExternalInput")
    d_diag = nc.dram_tensor("diag16", [128, 16], F32, kind="ExternalInput")
    d_ident = nc.dram_tensor("ident", [128, 128], F32, kind="ExternalInput")

    o_ems = nc.dram_tensor("ems_raw", [NH * 128, C], F32, kind="ExternalOutput")
    o_tc = nc.dram_tensor("tc_out", [BL, M], F32, kind="ExternalOutput")
    o_xs = nc.dram_tensor("xs_out", [BL, M], F32, kind="ExternalOutput")
    o_ce = nc.dram_tensor("ce_part", [NH, 128], F32, kind="ExternalOutput")
    o_ch = nc.dram_tensor("child_part", [NSG, JL], F32, kind="ExternalOutput")
    o_cf = nc.dram_tensor("conf_part", [NSG, JL], F32, kind="ExternalOutput")

    with TileContext(nc) as tc, ExitStack() as ctx:
        ep = ctx.enter_context
        xw_pool = ep(tc.tile_pool(name="xw", bufs=3))
        es_pool = ep(tc.tile_pool(name="es", bufs=3))
        col_pool = ep(tc.tile_pool(name="cols", bufs=2))
        sm_pool = ep(tc.tile_pool(name="sm", bufs=2))
        scr_pool = ep(tc.tile_pool(name="scr", bufs=10))
        w_pool = ep(tc.tile_pool(name="w", bufs=2))
        st_pool = ep(tc.tile_pool(name="st", bufs=2))
        idx_pool = ep(tc.tile_pool(name="idx", bufs=2))
        cst_pool = ep(tc.tile_pool(name="cst", bufs=1))
        ps_t = ep(tc.tile_pool(name="pst", bufs=2, space="PSUM"))
        ps_mm = ep(tc.tile_pool(name="psmm", bufs=4, space="PSUM"))

        nc.gpsimd.load_library(library_config.ap_gather)

        t_pat8 = cst_pool.tile([128, 8], F32)
        nc.sync.dma_start(t_pat8[:], d_pat8[:, :])
        t_diag = cst_pool.tile([128, 16], F32)
        nc.sync.dma_start(t_diag[:], d_diag[:, :])
        t_ident = cst_pool.tile([128, 128], F32)
        nc.sync.dma_start(t_ident[:], d_ident[:, :])

        for sg in range(NSG):
            se_all = col_pool.tile([128, JL], F32, tag="se_all")
            xlab_all = col_pool.tile([128, JL], F32, tag="xlab_all")
            xps = []

            # ---- Phase A: stream pair tiles: exp+sums and label gather ----
            for pl in range(PPS):
                pp = PPS * sg + pl
                xp = xw_pool.tile([128, 32 * C], xdt, tag="xw")
                xps.append(xp)
                nc.sync.dma_start(
                    xp[:].rearrange("p (t c) -> p t c", t=32),
                    d_x[4096 * pp:4096 * (pp + 1), :].rearrange(
                        "(t p) c -> p t c", p=128),
                )
                xpf = xp[:].bitcast(F32) if USE_F32R else xp[:]
                for h in range(4):
                    es = es_pool.tile([128, 8 * CP], F32, tag="es")
                    nc.vector.memset(
                        es[:].rearrange("p (t c) -> p t c", t=8)[:, :, C:CP],
                        0.0)
                    nc.scalar.activation(
                        es[:].rearrange("p (t c) -> p t c", t=8)[:, :, 0:C],
                        xpf[:, 8 * C * h:8 * C * (h + 1)].rearrange(
                            "p (t c) -> p t c", t=8),
                        ACTF.Exp,
                    )
                    for tt in range(8):
                        t = 8 * h + tt
                        nc.vector.tensor_scalar(
                            es[:, CP * tt:CP * (tt + 1)],
                            es[:, CP * tt:CP * (tt + 1)],
                            1.0, None, ALU.mult, ALU.add,
                            accum_out=se_all[:, 32 * pl + t:32 * pl + t + 1],
                        )
                gi = idx_pool.tile([128, 2], I16, tag="gi")
                nc.sync.dma_start(gi[:], d_gidx[pp, :, :])
                nc.gpsimd.ap_gather(
                    xlab_all[:, 32 * pl:32 * (pl + 1)].unsqueeze(2),
                    xpf.unsqueeze(2), gi[:],
                    channels=128, num_elems=32 * C, d=1, num_idxs=32,
                )

            # ---- Phase B: transpose to smalls layout [JL, 128] ----
            pt1 = ps_t.tile([JL, 128], F32, tag="pt")
            nc.tensor.transpose(pt1[:], se_all[:], t_ident[:])
            seT = scr_pool.tile([JL, 128], F32, tag="scr")
            nc.vector.tensor_copy(seT[:], pt1[:])
            pt2 = ps_t.tile([JL, 128], F32, tag="pt")
            nc.tensor.transpose(pt2[:], xlab_all[:], t_ident[:])
            xlabT = scr_pool.tile([JL, 128], F32, tag="scr")
            nc.vector.tensor_copy(xlabT[:], pt2[:])

            def seg(ap):  # [JL, 128] -> [JL, 8, 16]
                return ap.rearrange("p (a b) -> p a b", a=8)

            def bc(small):  # [JL, 8] -> [JL, 8, 16] step-0 broadcast
                return small.unsqueeze(2).broadcast_to([JL, 8, 16])

            # ---- Phase C: smalls ----
            teT = scr_pool.tile([JL, 128], F32, tag="scr")
            nc.scalar.activation(teT[:], xlabT[:], ACTF.Exp)
            rse = scr_pool.tile([JL, 128], F32, tag="scr")
            nc.vector.reciprocal(rse[:], seT[:])
            tp = scr_pool.tile([JL, 128], F32, tag="scr")
            nc.vector.tensor_mul(tp[:], teT[:], rse[:])
            lnp = sm_pool.tile([JL, 128], F32, tag="lnp")
            nc.scalar.activation(lnp[:], tp[:], ACTF.Ln)

            e1 = scr_pool.tile([JL, 128], F32, tag="scr")
            nc.scalar.activation(e1[:], tp[:], ACTF.Exp)
            s1 = scr_pool.tile([JL, 8], F32, tag="scr8")
            nc.vector.reduce_sum(s1[:], seg(e1[:]), axis=AX.X)
            r1 = scr_pool.tile([JL, 8], F32, tag="scr8")
            nc.vector.reciprocal(r1[:], s1[:])
            tcf = sm_pool.tile([JL, 128], F32, tag="tcf")
            nc.vector.tensor_tensor(seg(tcf[:]), seg(e1[:]), bc(r1[:]),
                                    ALU.mult)

            m8 = sm_pool.tile([JL, 64], F32, tag="m8")
            for q in range(8):
                nc.vector.max(m8[:, 8 * q:8 * (q + 1)],
                              tcf[:, 16 * q:16 * (q + 1)])
            thr = m8[:].rearrange("p (q e) -> p q e", q=8)[:, :, 3:4]
            gmask = scr_pool.tile([JL, 128], F32, tag="scr")
            nc.vector.tensor_tensor(
                seg(gmask[:]), seg(tcf[:]), thr.broadcast_to([JL, 8, 16]),
                ALU.is_ge)
            post = scr_pool.tile([JL, 128], F32, tag="scr")
            nc.vector.tensor_mul(post[:], tcf[:], gmask[:])
            sp = scr_pool.tile([JL, 8], F32, tag="scr8")
            nc.vector.reduce_sum(sp[:], seg(post[:]), axis=AX.X)
            rp = scr_pool.tile([JL, 8], F32, tag="scr8")
            nc.vector.reciprocal(rp[:], sp[:])
            pw = sm_pool.tile([JL, 128], F32, tag="pw")
            nc.vector.tensor_tensor(seg(pw[:]), seg(post[:]), bc(rp[:]),
                                    ALU.mult)

            stc = scr_pool.tile([JL, 8], F32, tag="scr8")
            nc.vector.reduce_sum(stc[:], seg(tcf[:]), axis=AX.X)
            rstc = scr_pool.tile([JL, 8], F32, tag="scr8")
            nc.vector.reciprocal(rstc[:], stc[:])
            wm = sm_pool.tile([JL, 128], F32, tag="wm")
            nc.vector.tensor_tensor(seg(wm[:]), seg(tcf[:]), bc(rstc[:]),
                                    ALU.mult)

            def softmax16(src, tag):
                e = scr_pool.tile([JL, 128], F32, tag="scr")
                nc.scalar.activation(e[:], src[:], ACTF.Exp)
                s = scr_pool.tile([JL, 8], F32, tag="scr8")
                nc.vector.reduce_sum(s[:], seg(e[:]), axis=AX.X)
                r = scr_pool.tile([JL, 8], F32, tag="scr8")
                nc.vector.reciprocal(r[:], s[:])
                o = sm_pool.tile([JL, 128], F32, tag=tag + "o")
                nc.vector.tensor_tensor(seg(o[:]), seg(e[:]), bc(r[:]),
                                        ALU.mult)
                return o

            xs = softmax16(wm, "xs")
            ts = softmax16(tcf, "ts")

            cexp = scr_pool.tile([JL, 128], F32, tag="scr")
            nc.scalar.activation(cexp[:], xs[:], ACTF.Exp, scale=-1.0)
            c2 = scr_pool.tile([JL, 128], F32, tag="scr")
            nc.scalar.activation(c2[:], cexp[:], ACTF.Ln, bias=1.0)
            c3 = scr_pool.tile([JL, 128], F32, tag="scr")
            nc.vector.scalar_tensor_tensor(
                out=c3[:], in0=ts[:], scalar=1.0, in1=xs[:],
                op0=ALU.subtract, op1=ALU.mult)   # (ts-1)*xs
            cel = scr_pool.tile([JL, 128], F32, tag="scr")
            cfr = sm_pool.tile([JL, 1], F32, tag="scr8")
            nc.vector.tensor_sub(cel[:], c2[:], c3[:])
            nc.vector.reduce_sum(cfr[:], cel[:], axis=AX.X)
            nc.sync.dma_start(o_cf[sg, :].unsqueeze(1), cfr[:])

            chl = scr_pool.tile([JL, 128], F32, tag="scr")
            chr_ = sm_pool.tile([JL, 1], F32, tag="scr8")
            nc.vector.tensor_mul(chl[:], lnp[:], xs[:])
            nc.vector.reduce_sum(chr_[:], chl[:], axis=AX.X)
            nc.sync.dma_start(o_ch[sg, :].unsqueeze(1), chr_[:])

            nc.sync.dma_start(
                o_tc[256 * sg:256 * (sg + 1), :].rearrange(
                    "(j r) m -> j (r m)", r=8), tcf[:])
            nc.sync.dma_start(
                o_xs[256 * sg:256 * (sg + 1), :].rearrange(
                    "(j r) m -> j (r m)", r=8), xs[:])

            # ---- Phase D: transpose weights back + build padded W ----
            pt3 = ps_t.tile([128, JL], F32, tag="ptb")
            nc.tensor.transpose(pt3[:], wm[:], t_ident[0:JL, 0:JL])
            wmT = st_pool.tile([128, JL], F32, tag="wmT")
            nc.vector.tensor_copy(wmT[:], pt3[:])
            pt4 = ps_t.tile([128, JL], F32, tag="ptb")
            nc.tensor.transpose(pt4[:], pw[:], t_ident[0:JL, 0:JL])
            pwT = st_pool.tile([128, JL], F32, tag="pwT")
            nc.vector.tensor_copy(pwT[:], pt4[:])

            # W layout: per B-tile j a 32-col block; real 16 cols at offset
            # 16*(j%2): [8 wm block-diag | 8 pw block-diag].
            wdt = F32R if USE_F32R else F32
            w2 = w_pool.tile([128, 32 * JL], wdt, tag="w2")
            w2f = w2[:].bitcast(F32) if USE_F32R else w2[:]
            nc.vector.memset(w2f, 0.0)
            for srcT, off in ((wmT, 0), (pwT, 8)):
                shifted = w2f[:, off:]
                out_view = bass.AP(
                    tensor=shifted.tensor, offset=shifted.offset,
                    ap=[list(shifted.ap[0]), [64, JL // 2], [48, 2], [1, 8]],
                )
                nc.vector.tensor_tensor(
                    out_view,
                    srcT[:].rearrange("p (a b) -> p a b", a=JL // 2)
                        .unsqueeze(3).broadcast_to([128, JL // 2, 2, 8]),
                    t_pat8[:].unsqueeze(1).unsqueeze(1)
                        .broadcast_to([128, JL // 2, 2, 8]),
                    ALU.mult,
                )

            # ---- Phase E: matmuls + ensemble CE per half-A-tile ----
            for hh in range(NH // NSG):
                hg = (NH // NSG) * sg + hh      # global half-A-tile 0..16
                pl = hh // 4                     # pair tile within sg
                xp = xps[pl]
                xpm = xp[:] if not USE_F32R else xp[:]
                psum = ps_mm.tile([128, 512], F32, tag="ps")
                for k in range(8):
                    tloc = 8 * (hh % 4) + k      # B-tile within pair
                    jl = 32 * pl + tloc          # W col-block within sg
                    v, s = k // 2, k % 2
                    nc.tensor.matmul(
                        psum[32 * v:32 * (v + 1), 0:C],
                        w2[:, 32 * jl:32 * (jl + 1)],
                        xpm[:, C * tloc:C * (tloc + 1)],
                        start=(s == 0), stop=(s == 1),
                        tile_position=(0, 32 * v),
                    )
                # ensemble CE from the w rows (p rows computed too, unused)
                esl = st_pool.tile([128, 2], F32, tag="esl")
                eEW = st_pool.tile([128, C], F32, tag="eEW")
                nc.scalar.activation(eEW[:], psum[:, 0:C], ACTF.Exp,
                                     accum_out=esl[:, 1:2])
                li = idx_pool.tile([128, 1], I16, tag="li")
                nc.sync.dma_start(li[:], d_lidx[hg, :].unsqueeze(1))
                gth = st_pool.tile([128, 16], F32, tag="gth")
                nc.gpsimd.ap_gather(
                    gth[:].unsqueeze(2), eEW[:].unsqueeze(2), li[:],
                    channels=128, num_elems=C, d=1, num_idxs=16,
                )
                gscr = st_pool.tile([128, 16], F32, tag="gscr")
                nc.vector.tensor_mul(gscr[:], gth[:], t_diag[:])
                nc.vector.reduce_sum(esl[:, 0:1], gscr[:], axis=AX.X)
                lncols = st_pool.tile([128, 2], F32, tag="lncols")
                nc.scalar.activation(lncols[:], esl[:], ACTF.Ln)
                cet = st_pool.tile([128, 1], F32, tag="cet")
                nc.vector.tensor_sub(cet[:], lncols[:, 1:2], lncols[:, 0:1])
                nc.sync.dma_start(o_ce[hg, :].unsqueeze(1), cet[:])

                pstage = st_pool.tile([128, C], F32, tag="pstage")
                nc.scalar.copy(pstage[:], psum[:, 0:C])
                nc.sync.dma_start(o_ems[128 * hg:128 * (hg + 1), :],
                                  pstage[:])

    nc.compile()
    return nc


def _host_inputs(y_pred, labels):
    """Build the 8 per-core input maps."""
    x = np.ascontiguousarray(
        np.asarray(y_pred, dtype=np.float32).reshape(B, M * C))
    lab = np.asarray(labels).astype(np.int64).reshape(B)

    p = np.arange(128)
    pat8 = (np.arange(8)[None, :] == (p // 16)[:, None]).astype(np.float32)
    diag16 = (np.arange(16)[None, :] == (p % 16)[:, None]).astype(np.float32)
    ident = np.eye(128, dtype=np.float32)

    in_maps = []
    for cidx in range(NCORES):
        xl = x[BL * cidx:BL * (cidx + 1)].reshape(PAIRS, C)
        ll = lab[BL * cidx:BL * (cidx + 1)]
        gidx = np.zeros((NPAIR, 128, 2), np.int16)
        for pp in range(NPAIR):
            for w in range(2):
                i = (p % 16) + 16 * w
                q = p // 16
                gidx[pp, :, w] = (C * i + ll[256 * pp + 8 * i + q]).astype(
                    np.int16)
        lidx = np.zeros((NH, 128), np.int16)
        for hg in range(NH):
            a, hh = hg // 2, hg % 2
            lidx[hg] = ll[128 * a + 64 * hh + 8 * (p // 16) + (p % 8)].astype(
                np.int16)
        in_maps.append({
            "x": xl, "gidx": gidx, "lidx": lidx,
            "pat8": pat8, "diag16": diag16, "ident": ident,
        })
    return in_maps


def _host_finalize(results):
    ems = np.empty((B, C), np.float32)
    tcf = np.concatenate([r["tc_out"] for r in results], 0)
    xs = np.concatenate([r["xs_out"] for r in results], 0)
    ce = np.float64(0.0)
    ch = np.float64(0.0)
    cf = np.float64(0.0)
    for cidx, r in enumerate(results):
        # ems_raw rows: [hg (16), k (8), wp (2), b_off (8)] -> w rows wp=0
        raw = r["ems_raw"].reshape(NH, 8, 2, 8, C)
        # row b (within core) = 128a + 64hh + 8k + b_off ; hg = 2a+hh
        emsc = raw[:, :, 1, :, :]          # p rows  [NH, 8, 8, C]
        ems[BL * cidx:BL * (cidx + 1)] = emsc.reshape(BL, C)
        cep = r["ce_part"].reshape(NH, 8, 2, 8)
        ce += np.sum(cep[:, :, 0, :], dtype=np.float64)
        ch += np.sum(r["child_part"], dtype=np.float64)
        cf += np.sum(r["conf_part"], dtype=np.float64)
    child_loss = np.float32(-ch / (B * M))
    conf_loss = np.float32(cf / (B * M))
    ens_loss = np.float32(ce / B)
    return (ems, child_loss, conf_loss, ens_loss, xs, tcf)


_NC_CACHE = None


def kernel(y_pred, labels, class_type, model_num, num_local):
    assert int(class_type) == C and int(model_num) == M and int(num_local) == K
    global _NC_CACHE
    if _NC_CACHE is None:
        _NC_CACHE = build_nc()
    nc = _NC_CACHE
    in_maps = _host_inputs(y_pred, labels)

    from concourse.bass_utils import run_bass_kernel_spmd
    res = run_bass_kernel_spmd(
        nc, in_maps, list(range(NCORES)),
        trace=bool(int(os.environ.get("KERNEL_TRACE", "0"))),
    )
    out = _host_finalize(res.results)
    if res.exec_time_ns is not None:
        kernel.last_exec_time_ns = res.exec_time_ns
        kernel.last_mean_exec_time_ns = res.mean_exec_time_ns
    return out


# revision 18
# speedup vs baseline: 1.0147x; 1.0147x over previous
"""Trainium2 Bass kernel for nn_Ensemble_of_ensemble (topk_masking).

Pure data-parallel over batch: 8192 rows split across 8 NeuronCores
(1024 rows each). Per core:
  - x viewed as [16384 (row,model) pairs, 345 classes], streamed as two
    [128, 32*345] "pair tiles" per super-group
  - per-pair-row softmax sum (se) via big ACT exp + DVE tensor_scalar accum
  - label logits gathered with GPSIMD ap_gather (per-16-partition groups
    share a row -> per-group index lists)
  - per-row model softmax / top-4 threshold (DVE max8) / L1 norms in a
    transposed [64 B-tiles, 128 (row_off, model)] smalls layout
  - both ensemble sums via fp32 TensorE matmuls with block-diagonal
    weights; w/p interleaved 16+16 per B-tile, packed 8 B-tiles per PSUM
    bank using 32-strip col tiling
  - ensemble CE via ACT exp+accum directly from PSUM + ap_gather
Scalar losses are finalized on host from per-row partial sums.
"""
import os
import numpy as np
from contextlib import ExitStack

import concourse.bass as bass
import concourse.bacc as bacc
import concourse.mybir as mybir
from concourse.tile import TileContext
from concourse import library_config

F32 = mybir.dt.float32
F32R = mybir.dt.float32r
I16 = mybir.dt.int16
ALU = mybir.AluOpType
ACTF = mybir.ActivationFunctionType
AX = mybir.AxisListType

B, M, C, K = 8192, 16, 345, 4
NCORES = 8
BL = B // NCORES            # 1024 rows per core
PAIRS = BL * M              # 16384 (row, model) pairs per core
NPAIR = 4                   # pair-tiles per core ([128, 32*345])
NSG = 4                     # super groups (one pair-tile each)
PPS = NPAIR // NSG          # pair-tiles per super group (1)
JL = 32                     # B-tiles per super group
NH = 16                     # half-A-tiles per core (psum groups of 8 B-tiles)
CP = C + 1                  # padded class stride (346, even)

USE_F32R = bool(int(os.environ.get("KERNEL_F32R", "0")))


def _patch_act_tables():
    # Exp and Ln interleave heavily; restrict the table-set choices to ones
    # containing BOTH so insert_act_table_loads emits a single load instead
    # of thrashing between exp_and_others and natural_log (~2.7us each).
    import concourse.hw_specs as hw_specs
    if getattr(bacc, "_act_tables_patched", False):
        return
    orig = bacc.get_activation_tables

    def filtered(arch):
        # Keep ALL sets in original order (act_func_set_id is the index
        # into act_info.json) but hide Exp/Ln from every set that doesn't
        # contain both, so the load-insertion pass always picks the
        # combined set for them.
        tabs = orig(arch)
        need = {ACTF.Exp, ACTF.Ln}
        if not any(need <= v for v in tabs.values()):
            return tabs
        out = {}
        for k, v in tabs.items():
            out[k] = set(v) if need <= v else set(v) - need
        return out

    bacc.get_activation_tables = filtered
    bacc._act_tables_patched = True


def build_nc():
    _patch_act_tables()
    nc = bacc.Bacc("TRN2", target_bir_lowering=False, debug=False,
                   num_devices=NCORES)

    xdt = F32R if USE_F32R else F32
    d_x = nc.dram_tensor("x", [PAIRS, C], xdt, kind="ExternalInput")
    d_gidx = nc.dram_tensor("gidx", [NPAIR, 128, 2], I16, kind="ExternalInput")
    d_lidx = nc.dram_tensor("lidx", [NH, 128], I16, kind="ExternalInput")
    d_pat8 = nc.dram_tensor("pat8", [128, 8], F32, kind="ExternalInput")
    d_diag = nc.dram_tensor("diag16", [128, 16], F32, kind="ExternalInput")
    d_ident = nc.dram_tensor("ident", [128, 128], F32, kind="ExternalInput")

    o_ems = nc.dram_tensor("ems_raw", [NH * 128, C], F32, kind="ExternalOutput")
    o_tc = nc.dram_tensor("tc_out", [BL, M], F32, kind="ExternalOutput")
    o_xs = nc.dram_tensor("xs_out", [BL, M], F32, kind="ExternalOutput")
    o_ce = nc.dram_tensor("ce_part", [NH, 128], F32, kind="ExternalOutput")
    o_ch = nc.dram_tensor("child_part", [NSG, JL], F32, kind="ExternalOutput")
    o_cf = nc.dram_tensor("conf_part", [NSG, JL], F32, kind="ExternalOutput")

    with TileContext(nc) as tc, ExitStack() as ctx:
        ep = ctx.enter_context
        xw_pool = ep(tc.tile_pool(name="xw", bufs=3))
        es_pool = ep(tc.tile_pool(name="es", bufs=3))
        col_pool = ep(tc.tile_pool(name="cols", bufs=2))
        sm_pool = ep(tc.tile_pool(name="sm", bufs=2))
        scr_pool = ep(tc.tile_pool(name="scr", bufs=10))
        w_pool = ep(tc.tile_pool(name="w", bufs=8))
        st_pool = ep(tc.tile_pool(name="st", bufs=2))
        idx_pool = ep(tc.tile_pool(name="idx", bufs=2))
        cst_pool = ep(tc.tile_pool(name="cst", bufs=1))
        ps_t = ep(tc.tile_pool(name="pst", bufs=2, space="PSUM"))
        ps_mm = ep(tc.tile_pool(name="psmm", bufs=4, space="PSUM"))

        nc.gpsimd.load_library(library_config.ap_gather)

        t_pat8 = cst_pool.tile([128, 8], F32)
        nc.sync.dma_start(t_pat8[:], d_pat8[:, :])
        t_diag = cst_pool.tile([128, 16], F32)
        nc.sync.dma_start(t_diag[:], d_diag[:, :])
        t_ident = cst_pool.tile([128, 128], F32)
        nc.sync.dma_start(t_ident[:], d_ident[:, :])

        for sg in range(NSG):
            se_all = col_pool.tile([128, JL], F32, tag="se_all")
            xlab_all = col_pool.tile([128, JL], F32, tag="xlab_all")
            xps = []

            # ---- Phase A: stream pair tiles: exp+sums and label gather ----
            for pl in range(PPS):
                pp = PPS * sg + pl
                xp = xw_pool.tile([128, 32 * C], xdt, tag="xw")
                xps.append(xp)
                nc.sync.dma_start(
                    xp[:].rearrange("p (t c) -> p t c", t=32),
                    d_x[4096 * pp:4096 * (pp + 1), :].rearrange(
                        "(t p) c -> p t c", p=128),
                )
                xpf = xp[:].bitcast(F32) if USE_F32R else xp[:]
                for h in range(4):
                    es = es_pool.tile([128, 8 * CP], F32, tag="es")
                    nc.vector.memset(
                        es[:].rearrange("p (t c) -> p t c", t=8)[:, :, C:CP],
                        0.0)
                    nc.scalar.activation(
                        es[:].rearrange("p (t c) -> p t c", t=8)[:, :, 0:C],
                        xpf[:, 8 * C * h:8 * C * (h + 1)].rearrange(
                            "p (t c) -> p t c", t=8),
                        ACTF.Exp,
                    )
                    for tt in range(8):
                        t = 8 * h + tt
                        nc.vector.tensor_scalar(
                            es[:, CP * tt:CP * (tt + 1)],
                            es[:, CP * tt:CP * (tt + 1)],
                            1.0, None, ALU.mult, ALU.add,
                            accum_out=se_all[:, 32 * pl + t:32 * pl + t + 1],
                        )
                gi = idx_pool.tile([128, 2], I16, tag="gi")
                nc.sync.dma_start(gi[:], d_gidx[pp, :, :])
                nc.gpsimd.ap_gather(
                    xlab_all[:, 32 * pl:32 * (pl + 1)].unsqueeze(2),
                    xpf.unsqueeze(2), gi[:],
                    channels=128, num_elems=32 * C, d=1, num_idxs=32,
                )

            # ---- Phase B: transpose to smalls layout [JL, 128] ----
            pt1 = ps_t.tile([JL, 128], F32, tag="pt")
            nc.tensor.transpose(pt1[:], se_all[:], t_ident[:])
            seT = scr_pool.tile([JL, 128], F32, tag="scr")
            nc.vector.tensor_copy(seT[:], pt1[:])
            pt2 = ps_t.tile([JL, 128], F32, tag="pt")
            nc.tensor.transpose(pt2[:], xlab_all[:], t_ident[:])
            xlabT = scr_pool.tile([JL, 128], F32, tag="scr")
            nc.vector.tensor_copy(xlabT[:], pt2[:])

            def seg(ap):  # [JL, 128] -> [JL, 8, 16]
                return ap.rearrange("p (a b) -> p a b", a=8)

            def bc(small):  # [JL, 8] -> [JL, 8, 16] step-0 broadcast
                return small.unsqueeze(2).broadcast_to([JL, 8, 16])

            # ---- Phase C: smalls ----
            teT = scr_pool.tile([JL, 128], F32, tag="scr")
            nc.scalar.activation(teT[:], xlabT[:], ACTF.Exp)
            rse = scr_pool.tile([JL, 128], F32, tag="scr")
            nc.vector.reciprocal(rse[:], seT[:])
            tp = scr_pool.tile([JL, 128], F32, tag="scr")
            nc.vector.tensor_mul(tp[:], teT[:], rse[:])
            lnp = sm_pool.tile([JL, 128], F32, tag="lnp")
            nc.scalar.activation(lnp[:], tp[:], ACTF.Ln)

            e1 = scr_pool.tile([JL, 128], F32, tag="scr")
            nc.scalar.activation(e1[:], tp[:], ACTF.Exp)
            s1 = scr_pool.tile([JL, 8], F32, tag="scr8")
            nc.vector.reduce_sum(s1[:], seg(e1[:]), axis=AX.X)
            r1 = scr_pool.tile([JL, 8], F32, tag="scr8")
            nc.vector.reciprocal(r1[:], s1[:])
            tcf = sm_pool.tile([JL, 128], F32, tag="tcf")
            nc.vector.tensor_tensor(seg(tcf[:]), seg(e1[:]), bc(r1[:]),
                                    ALU.mult)

            m8 = sm_pool.tile([JL, 64], F32, tag="m8")
            for q in range(8):
                nc.vector.max(m8[:, 8 * q:8 * (q + 1)],
                              tcf[:, 16 * q:16 * (q + 1)])
            thr = m8[:].rearrange("p (q e) -> p q e", q=8)[:, :, 3:4]
            gmask = scr_pool.tile([JL, 128], F32, tag="scr")
            nc.vector.tensor_tensor(
                seg(gmask[:]), seg(tcf[:]), thr.broadcast_to([JL, 8, 16]),
                ALU.is_ge)
            post = scr_pool.tile([JL, 128], F32, tag="scr")
            nc.vector.tensor_mul(post[:], tcf[:], gmask[:])
            sp = scr_pool.tile([JL, 8], F32, tag="scr8")
            nc.vector.reduce_sum(sp[:], seg(post[:]), axis=AX.X)
            rp = scr_pool.tile([JL, 8], F32, tag="scr8")
            nc.vector.reciprocal(rp[:], sp[:])
            pw = sm_pool.tile([JL, 128], F32, tag="pw")
            nc.vector.tensor_tensor(seg(pw[:]), seg(post[:]), bc(rp[:]),
                                    ALU.mult)

            stc = scr_pool.tile([JL, 8], F32, tag="scr8")
            nc.vector.reduce_sum(stc[:], seg(tcf[:]), axis=AX.X)
            rstc = scr_pool.tile([JL, 8], F32, tag="scr8")
            nc.vector.reciprocal(rstc[:], stc[:])
            wm = sm_pool.tile([JL, 128], F32, tag="wm")
            nc.vector.tensor_tensor(seg(wm[:]), seg(tcf[:]), bc(rstc[:]),
                                    ALU.mult)

            def softmax16(src, tag):
                e = scr_pool.tile([JL, 128], F32, tag="scr")
                nc.scalar.activation(e[:], src[:], ACTF.Exp)
                s = scr_pool.tile([JL, 8], F32, tag="scr8")
                nc.vector.reduce_sum(s[:], seg(e[:]), axis=AX.X)
                r = scr_pool.tile([JL, 8], F32, tag="scr8")
                nc.vector.reciprocal(r[:], s[:])
                o = sm_pool.tile([JL, 128], F32, tag=tag + "o")
                nc.vector.tensor_tensor(seg(o[:]), seg(e[:]), bc(r[:]),
                                        ALU.mult)
                return o

            xs = softmax16(wm, "xs")
            ts = softmax16(tcf, "ts")

            cexp = scr_pool.tile([JL, 128], F32, tag="scr")
            nc.scalar.activation(cexp[:], xs[:], ACTF.Exp, scale=-1.0)
            c2 = scr_pool.tile([JL, 128], F32, tag="scr")
            nc.scalar.activation(c2[:], cexp[:], ACTF.Ln, bias=1.0)
            c3 = scr_pool.tile([JL, 128], F32, tag="scr")
            nc.vector.scalar_tensor_tensor(
                out=c3[:], in0=ts[:], scalar=1.0, in1=xs[:],
                op0=ALU.subtract, op1=ALU.mult)   # (ts-1)*xs
            cel = scr_pool.tile([JL, 128], F32, tag="scr")
            cfr = sm_pool.tile([JL, 1], F32, tag="scr8")
            nc.vector.tensor_sub(cel[:], c2[:], c3[:])
            nc.vector.reduce_sum(cfr[:], cel[:], axis=AX.X)
            nc.sync.dma_start(o_cf[sg, :].unsqueeze(1), cfr[:])

            chl = scr_pool.tile([JL, 128], F32, tag="scr")
            chr_ = sm_pool.tile([JL, 1], F32, tag="scr8")
            nc.vector.tensor_mul(chl[:], lnp[:], xs[:])
            nc.vector.reduce_sum(chr_[:], chl[:], axis=AX.X)
            nc.sync.dma_start(o_ch[sg, :].unsqueeze(1), chr_[:])

            nc.sync.dma_start(
                o_tc[256 * sg:256 * (sg + 1), :].rearrange(
                    "(j r) m -> j (r m)", r=8), tcf[:])
            nc.sync.dma_start(
                o_xs[256 * sg:256 * (sg + 1), :].rearrange(
                    "(j r) m -> j (r m)", r=8), xs[:])

            # ---- Phase D: transpose weights back + build padded W ----
            pt3 = ps_t.tile([128, JL], F32, tag="ptb")
            nc.tensor.transpose(pt3[:], wm[:], t_ident[0:JL, 0:JL])
            wmT = st_pool.tile([128, JL], F32, tag="wmT")
            nc.vector.tensor_copy(wmT[:], pt3[:])
            pt4 = ps_t.tile([128, JL], F32, tag="ptb")
            nc.tensor.transpose(pt4[:], pw[:], t_ident[0:JL, 0:JL])
            pwT = st_pool.tile([128, JL], F32, tag="pwT")
            nc.vector.tensor_copy(pwT[:], pt4[:])

            # W layout: per B-tile j a 32-col block; real 16 cols at offset
            # 16*(j%2): [8 wm block-diag | 8 pw block-diag].
            wdt = F32R if USE_F32R else F32
            # four independent 8-block W tiles so each psum group's matmuls
            # start as soon as its own slice is built (whole-tile deps)
            w2cs = []
            for gch in range(4):
                w2c = w_pool.tile([128, 8 * 32], wdt, tag="w2")
                w2cs.append(w2c)
                w2cf = w2c[:].bitcast(F32) if USE_F32R else w2c[:]
                nc.vector.memset(w2cf, 0.0)
                for srcT, off in ((wmT, 0), (pwT, 8)):
                    shifted = w2cf[:, off:]
                    out_view = bass.AP(
                        tensor=shifted.tensor, offset=shifted.offset,
                        ap=[list(shifted.ap[0]), [64, 4], [48, 2], [1, 8]],
                    )
                    nc.vector.tensor_tensor(
                        out_view,
                        srcT[:, 8 * gch:8 * (gch + 1)]
                            .rearrange("p (a b) -> p a b", a=4)
                            .unsqueeze(3).broadcast_to([128, 4, 2, 8]),
                        t_pat8[:].unsqueeze(1).unsqueeze(1)
                            .broadcast_to([128, 4, 2, 8]),
                        ALU.mult,
                    )

            # ---- Phase E: matmuls + ensemble CE per half-A-tile ----
            for hh in range(NH // NSG):
                hg = (NH // NSG) * sg + hh      # global half-A-tile 0..16
                pl = hh // 4                     # pair tile within sg
                xp = xps[pl]
                xpm = xp[:] if not USE_F32R else xp[:]
                psum = ps_mm.tile([128, 512], F32, tag="ps")
                for k in range(8):
                    tloc = 8 * (hh % 4) + k      # B-tile within pair
                    jl = 32 * pl + tloc          # W col-block within sg
                    v, s = k // 2, k % 2
                    nc.tensor.matmul(
                        psum[32 * v:32 * (v + 1), 0:C],
                        w2cs[hh][:, 32 * k:32 * (k + 1)],
                        xpm[:, C * tloc:C * (tloc + 1)],
                        start=(s == 0), stop=(s == 1),
                        tile_position=(0, 32 * v),
                    )
                # ensemble CE from the w rows (p rows computed too, unused)
                esl = st_pool.tile([128, 2], F32, tag="esl")
                eEW = st_pool.tile([128, C], F32, tag="eEW")
                nc.scalar.activation(eEW[:], psum[:, 0:C], ACTF.Exp,
                                     accum_out=esl[:, 1:2])
                li = idx_pool.tile([128, 1], I16, tag="li")
                nc.sync.dma_start(li[:], d_lidx[hg, :].unsqueeze(1))
                gth = st_pool.tile([128, 16], F32, tag="gth")
                nc.gpsimd.ap_gather(
                    gth[:].unsqueeze(2), eEW[:].unsqueeze(2), li[:],
                    channels=128, num_elems=C, d=1, num_idxs=16,
                )
                gscr = st_pool.tile([128, 16], F32, tag="gscr")
                nc.vector.tensor_mul(gscr[:], gth[:], t_diag[:])
                nc.vector.reduce_sum(esl[:, 0:1], gscr[:], axis=AX.X)
                lncols = st_pool.tile([128, 2], F32, tag="lncols")
                nc.scalar.activation(lncols[:], esl[:], ACTF.Ln)
                cet = st_pool.tile([128, 1], F32, tag="cet")
                nc.vector.tensor_sub(cet[:], lncols[:, 1:2], lncols[:, 0:1])
                nc.sync.dma_start(o_ce[hg, :].unsqueeze(1), cet[:])

                pstage = st_pool.tile([128, C], F32, tag="pstage")
                nc.scalar.copy(pstage[:], psum[:, 0:C])
                nc.sync.dma_start(o_ems[128 * hg:128 * (hg + 1), :],
                                  pstage[:])

    nc.compile()
    return nc


def _host_inputs(y_pred, labels):
    """Build the 8 per-core input maps."""
    x = np.ascontiguousarray(
        np.asarray(y_pred, dtype=np.float32).reshape(B, M * C))
    lab = np.asarray(labels).astype(np.int64).reshape(B)

    p = np.arange(128)
    pat8 = (np.arange(8)[None, :] == (p // 16)[:, None]).astype(np.float32)
    diag16 = (np.arange(16)[None, :] == (p % 16)[:, None]).astype(np.float32)
    ident = np.eye(128, dtype=np.float32)

    in_maps = []
    for cidx in range(NCORES):
        xl = x[BL * cidx:BL * (cidx + 1)].reshape(PAIRS, C)
        ll = lab[BL * cidx:BL * (cidx + 1)]
        gidx = np.zeros((NPAIR, 128, 2), np.int16)
        for pp in range(NPAIR):
            for w in range(2):
                i = (p % 16) + 16 * w
                q = p // 16
                gidx[pp, :, w] = (C * i + ll[256 * pp + 8 * i + q]).astype(
                    np.int16)
        lidx = np.zeros((NH, 128), np.int16)
        for hg in range(NH):
            a, hh = hg // 2, hg % 2
            lidx[hg] = ll[128 * a + 64 * hh + 8 * (p // 16) + (p % 8)].astype(
                np.int16)
        in_maps.append({
            "x": xl, "gidx": gidx, "lidx": lidx,
            "pat8": pat8, "diag16": diag16, "ident": ident,
        })
    return in_maps


def _host_finalize(results):
    ems = np.empty((B, C), np.float32)
    tcf = np.concatenate([r["tc_out"] for r in results], 0)
    xs = np.concatenate([r["xs_out"] for r in results], 0)
    ce = np.float64(0.0)
    ch = np.float64(0.0)
    cf = np.float64(0.0)
    for cidx, r in enumerate(results):
        # ems_raw rows: [hg (16), k (8), wp (2), b_off (8)] -> w rows wp=0
        raw = r["ems_raw"].reshape(NH, 8, 2, 8, C)
        # row b (within core) = 128a + 64hh + 8k + b_off ; hg = 2a+hh
        emsc = raw[:, :, 1, :, :]          # p rows  [NH, 8, 8, C]
        ems[BL * cidx:BL * (cidx + 1)] = emsc.reshape(BL, C)
        cep = r["ce_part"].reshape(NH, 8, 2, 8)
        ce += np.sum(cep[:, :, 0, :], dtype=np.float64)
        ch += np.sum(r["child_part"], dtype=np.float64)
        cf += np.sum(r["conf_part"], dtype=np.float64)
    child_loss = np.float32(-ch / (B * M))
    conf_loss = np.float32(cf / (B * M))
    ens_loss = np.float32(ce / B)
    return (ems, child_loss, conf_loss, ens_loss, xs, tcf)


_NC_CACHE = None


def kernel(y_pred, labels, class_type, model_num, num_local):
    assert int(class_type) == C and int(model_num) == M and int(num_local) == K
    global _NC_CACHE
    if _NC_CACHE is None:
        _NC_CACHE = build_nc()
    nc = _NC_CACHE
    in_maps = _host_inputs(y_pred, labels)

    from concourse.bass_utils import run_bass_kernel_spmd
    res = run_bass_kernel_spmd(
        nc, in_maps, list(range(NCORES)),
        trace=bool(int(os.environ.get("KERNEL_TRACE", "0"))),
    )
    out = _host_finalize(res.results)
    if res.exec_time_ns is not None:
        kernel.last_exec_time_ns = res.exec_time_ns
        kernel.last_mean_exec_time_ns = res.mean_exec_time_ns
    return out


# revision 21
# speedup vs baseline: 1.0737x; 1.0582x over previous
"""Trainium2 Bass kernel for nn_Ensemble_of_ensemble (topk_masking).

Pure data-parallel over batch: 8192 rows split across 8 NeuronCores
(1024 rows each). Per core:
  - x viewed as [16384 (row,model) pairs, 345 classes], streamed as two
    [128, 32*345] "pair tiles" per super-group
  - per-pair-row softmax sum (se) via big ACT exp + DVE tensor_scalar accum
  - label logits gathered with GPSIMD ap_gather (per-16-partition groups
    share a row -> per-group index lists)
  - per-row model softmax / top-4 threshold (DVE max8) / L1 norms in a
    transposed [64 B-tiles, 128 (row_off, model)] smalls layout
  - both ensemble sums via fp32 TensorE matmuls with block-diagonal
    weights; w/p interleaved 16+16 per B-tile, packed 8 B-tiles per PSUM
    bank using 32-strip col tiling
  - ensemble CE via ACT exp+accum directly from PSUM + ap_gather
Scalar losses are finalized on host from per-row partial sums.
"""
import os
import numpy as np
from contextlib import ExitStack

import concourse.bass as bass
import concourse.bacc as bacc
import concourse.mybir as mybir
from concourse.tile import TileContext
from concourse import library_config

F32 = mybir.dt.float32
F32R = mybir.dt.float32r
I16 = mybir.dt.int16
ALU = mybir.AluOpType
ACTF = mybir.ActivationFunctionType
AX = mybir.AxisListType

B, M, C, K = 8192, 16, 345, 4
NCORES = 8
BL = B // NCORES            # 1024 rows per core
PAIRS = BL * M              # 16384 (row, model) pairs per core
NPAIR = 4                   # pair-tiles per core ([128, 32*345])
NSG = 4                     # super groups (one pair-tile each)
PPS = NPAIR // NSG          # pair-tiles per super group (1)
JL = 32                     # B-tiles per super group
NH = 16                     # half-A-tiles per core (psum groups of 8 B-tiles)
CP = C + 1                  # padded class stride (346, even)

USE_F32R = bool(int(os.environ.get("KERNEL_F32R", "0")))


def _patch_act_tables():
    # Exp and Ln interleave heavily; restrict the table-set choices to ones
    # containing BOTH so insert_act_table_loads emits a single load instead
    # of thrashing between exp_and_others and natural_log (~2.7us each).
    import concourse.hw_specs as hw_specs
    if getattr(bacc, "_act_tables_patched", False):
        return
    orig = bacc.get_activation_tables

    def filtered(arch):
        # Keep ALL sets in original order (act_func_set_id is the index
        # into act_info.json) but hide Exp/Ln from every set that doesn't
        # contain both, so the load-insertion pass always picks the
        # combined set for them.
        tabs = orig(arch)
        need = {ACTF.Exp, ACTF.Ln}
        if not any(need <= v for v in tabs.values()):
            return tabs
        out = {}
        for k, v in tabs.items():
            out[k] = set(v) if need <= v else set(v) - need
        return out

    bacc.get_activation_tables = filtered
    bacc._act_tables_patched = True


def build_nc():
    _patch_act_tables()
    nc = bacc.Bacc("TRN2", target_bir_lowering=False, debug=False,
                   num_devices=NCORES)

    xdt = F32R if USE_F32R else F32
    d_x = nc.dram_tensor("x", [PAIRS, C], xdt, kind="ExternalInput")
    d_gidx = nc.dram_tensor("gidx", [NPAIR, 128, 2], I16, kind="ExternalInput")
    d_lidx = nc.dram_tensor("lidx", [NH, 128], I16, kind="ExternalInput")
    d_pat8 = nc.dram_tensor("pat8", [128, 8], F32, kind="ExternalInput")
    d_diag = nc.dram_tensor("diag16", [128, 16], F32, kind="ExternalInput")
    d_ident = nc.dram_tensor("ident", [128, 128], F32, kind="ExternalInput")

    o_ems = nc.dram_tensor("ems_raw", [NH * 128, C], F32, kind="ExternalOutput")
    o_tc = nc.dram_tensor("tc_out", [BL, M], F32, kind="ExternalOutput")
    o_xs = nc.dram_tensor("xs_out", [BL, M], F32, kind="ExternalOutput")
    o_ce = nc.dram_tensor("ce_part", [NH, 128], F32, kind="ExternalOutput")
    o_ch = nc.dram_tensor("child_part", [NSG, JL], F32, kind="ExternalOutput")
    o_cf = nc.dram_tensor("conf_part", [NSG, JL], F32, kind="ExternalOutput")

    with TileContext(nc) as tc, ExitStack() as ctx:
        ep = ctx.enter_context
        xw_pool = ep(tc.tile_pool(name="xw", bufs=3))
        es_pool = ep(tc.tile_pool(name="es", bufs=3))
        col_pool = ep(tc.tile_pool(name="cols", bufs=2))
        sm_pool = ep(tc.tile_pool(name="sm", bufs=2))
        scr_pool = ep(tc.tile_pool(name="scr", bufs=10))
        w_pool = ep(tc.tile_pool(name="w", bufs=8))
        st_pool = ep(tc.tile_pool(name="st", bufs=2))
        idx_pool = ep(tc.tile_pool(name="idx", bufs=2))
        cst_pool = ep(tc.tile_pool(name="cst", bufs=1))
        ps_t = ep(tc.tile_pool(name="pst", bufs=2, space="PSUM"))
        ps_mm = ep(tc.tile_pool(name="psmm", bufs=4, space="PSUM"))

        nc.gpsimd.load_library(library_config.ap_gather)

        t_pat8 = cst_pool.tile([128, 8], F32)
        nc.sync.dma_start(t_pat8[:], d_pat8[:, :])
        t_diag = cst_pool.tile([128, 16], F32)
        nc.sync.dma_start(t_diag[:], d_diag[:, :])
        t_ident = cst_pool.tile([128, 128], F32)
        nc.sync.dma_start(t_ident[:], d_ident[:, :])

        for sg in range(NSG):
            se_all = col_pool.tile([128, JL], F32, tag="se_all")
            xlab_all = col_pool.tile([128, JL], F32, tag="xlab_all")
            xps = []

            # ---- Phase A: stream pair tiles: exp+sums and label gather ----
            for pl in range(PPS):
                pp = PPS * sg + pl
                xp = xw_pool.tile([128, 32 * C], xdt, tag="xw")
                xps.append(xp)
                nc.sync.dma_start(
                    xp[:].rearrange("p (t c) -> p t c", t=32),
                    d_x[4096 * pp:4096 * (pp + 1), :].rearrange(
                        "(t p) c -> p t c", p=128),
                )
                xpf = xp[:].bitcast(F32) if USE_F32R else xp[:]
                for h in range(4):
                    es = es_pool.tile([128, 8 * CP], F32, tag="es")
                    nc.vector.memset(
                        es[:].rearrange("p (t c) -> p t c", t=8)[:, :, C:CP],
                        0.0)
                    nc.scalar.activation(
                        es[:].rearrange("p (t c) -> p t c", t=8)[:, :, 0:C],
                        xpf[:, 8 * C * h:8 * C * (h + 1)].rearrange(
                            "p (t c) -> p t c", t=8),
                        ACTF.Exp,
                    )
                    for tt in range(8):
                        t = 8 * h + tt
                        nc.vector.tensor_scalar(
                            es[:, CP * tt:CP * (tt + 1)],
                            es[:, CP * tt:CP * (tt + 1)],
                            1.0, None, ALU.mult, ALU.add,
                            accum_out=se_all[:, 32 * pl + t:32 * pl + t + 1],
                        )
                gi = idx_pool.tile([128, 2], I16, tag="gi")
                nc.sync.dma_start(gi[:], d_gidx[pp, :, :])
                nc.gpsimd.ap_gather(
                    xlab_all[:, 32 * pl:32 * (pl + 1)].unsqueeze(2),
                    xpf.unsqueeze(2), gi[:],
                    channels=128, num_elems=32 * C, d=1, num_idxs=32,
                )

            # ---- Phase B: transpose to smalls layout [JL, 128] ----
            pt1 = ps_t.tile([JL, 128], F32, tag="pt")
            nc.tensor.transpose(pt1[:], se_all[:], t_ident[:])
            seT = scr_pool.tile([JL, 128], F32, tag="scr")
            nc.vector.tensor_copy(seT[:], pt1[:])
            pt2 = ps_t.tile([JL, 128], F32, tag="pt")
            nc.tensor.transpose(pt2[:], xlab_all[:], t_ident[:])
            xlabT = scr_pool.tile([JL, 128], F32, tag="scr")
            nc.vector.tensor_copy(xlabT[:], pt2[:])

            def seg(ap):  # [JL, 128] -> [JL, 8, 16]
                return ap.rearrange("p (a b) -> p a b", a=8)

            def bc(small):  # [JL, 8] -> [JL, 8, 16] step-0 broadcast
                return small.unsqueeze(2).broadcast_to([JL, 8, 16])

            # ---- Phase C: smalls ----
            teT = scr_pool.tile([JL, 128], F32, tag="scr")
            nc.scalar.activation(teT[:], xlabT[:], ACTF.Exp)
            rse = scr_pool.tile([JL, 128], F32, tag="scr")
            nc.vector.reciprocal(rse[:], seT[:])
            tp = scr_pool.tile([JL, 128], F32, tag="scr")
            nc.vector.tensor_mul(tp[:], teT[:], rse[:])
            lnp = sm_pool.tile([JL, 128], F32, tag="lnp")
            nc.scalar.activation(lnp[:], tp[:], ACTF.Ln)

            e1 = scr_pool.tile([JL, 128], F32, tag="scr")
            nc.scalar.activation(e1[:], tp[:], ACTF.Exp)
            s1 = scr_pool.tile([JL, 8], F32, tag="scr8")
            nc.vector.reduce_sum(s1[:], seg(e1[:]), axis=AX.X)
            r1 = scr_pool.tile([JL, 8], F32, tag="scr8")
            nc.vector.reciprocal(r1[:], s1[:])
            tcf = sm_pool.tile([JL, 128], F32, tag="tcf")
            nc.vector.tensor_tensor(seg(tcf[:]), seg(e1[:]), bc(r1[:]),
                                    ALU.mult)

            m8 = sm_pool.tile([JL, 64], F32, tag="m8")
            for q in range(8):
                nc.vector.max(m8[:, 8 * q:8 * (q + 1)],
                              tcf[:, 16 * q:16 * (q + 1)])
            thr = m8[:].rearrange("p (q e) -> p q e", q=8)[:, :, 3:4]
            gmask = scr_pool.tile([JL, 128], F32, tag="scr")
            nc.vector.tensor_tensor(
                seg(gmask[:]), seg(tcf[:]), thr.broadcast_to([JL, 8, 16]),
                ALU.is_ge)
            post = scr_pool.tile([JL, 128], F32, tag="scr")
            nc.vector.tensor_mul(post[:], tcf[:], gmask[:])
            sp = scr_pool.tile([JL, 8], F32, tag="scr8")
            nc.vector.reduce_sum(sp[:], seg(post[:]), axis=AX.X)
            rp = scr_pool.tile([JL, 8], F32, tag="scr8")
            nc.vector.reciprocal(rp[:], sp[:])
            pw = sm_pool.tile([JL, 128], F32, tag="pw")
            nc.vector.tensor_tensor(seg(pw[:]), seg(post[:]), bc(rp[:]),
                                    ALU.mult)


            def softmax16(src, tag):
                e = scr_pool.tile([JL, 128], F32, tag="scr")
                nc.scalar.activation(e[:], src[:], ACTF.Exp)
                s = scr_pool.tile([JL, 8], F32, tag="scr8")
                nc.vector.reduce_sum(s[:], seg(e[:]), axis=AX.X)
                r = scr_pool.tile([JL, 8], F32, tag="scr8")
                nc.vector.reciprocal(r[:], s[:])
                o = sm_pool.tile([JL, 128], F32, tag=tag + "o")
                nc.vector.tensor_tensor(seg(o[:]), seg(e[:]), bc(r[:]),
                                        ALU.mult)
                return o

            xs = softmax16(tcf, "xs")
            ts = xs

            cexp = scr_pool.tile([JL, 128], F32, tag="scr")
            nc.scalar.activation(cexp[:], xs[:], ACTF.Exp, scale=-1.0)
            c2 = scr_pool.tile([JL, 128], F32, tag="scr")
            nc.scalar.activation(c2[:], cexp[:], ACTF.Ln, bias=1.0)
            c3 = scr_pool.tile([JL, 128], F32, tag="scr")
            nc.vector.scalar_tensor_tensor(
                out=c3[:], in0=ts[:], scalar=1.0, in1=xs[:],
                op0=ALU.subtract, op1=ALU.mult)   # (ts-1)*xs
            cel = scr_pool.tile([JL, 128], F32, tag="scr")
            cfr = sm_pool.tile([JL, 1], F32, tag="scr8")
            nc.vector.tensor_sub(cel[:], c2[:], c3[:])
            nc.vector.reduce_sum(cfr[:], cel[:], axis=AX.X)
            nc.sync.dma_start(o_cf[sg, :].unsqueeze(1), cfr[:])

            chl = scr_pool.tile([JL, 128], F32, tag="scr")
            chr_ = sm_pool.tile([JL, 1], F32, tag="scr8")
            nc.vector.tensor_mul(chl[:], lnp[:], xs[:])
            nc.vector.reduce_sum(chr_[:], chl[:], axis=AX.X)
            nc.sync.dma_start(o_ch[sg, :].unsqueeze(1), chr_[:])

            nc.sync.dma_start(
                o_tc[256 * sg:256 * (sg + 1), :].rearrange(
                    "(j r) m -> j (r m)", r=8), tcf[:])
            nc.sync.dma_start(
                o_xs[256 * sg:256 * (sg + 1), :].rearrange(
                    "(j r) m -> j (r m)", r=8), xs[:])

            # ---- Phase D: transpose weights back + build padded W ----
            pt3 = ps_t.tile([128, JL], F32, tag="ptb")
            nc.tensor.transpose(pt3[:], tcf[:], t_ident[0:JL, 0:JL])
            wmT = st_pool.tile([128, JL], F32, tag="wmT")
            nc.vector.tensor_copy(wmT[:], pt3[:])
            pt4 = ps_t.tile([128, JL], F32, tag="ptb")
            nc.tensor.transpose(pt4[:], pw[:], t_ident[0:JL, 0:JL])
            pwT = st_pool.tile([128, JL], F32, tag="pwT")
            nc.vector.tensor_copy(pwT[:], pt4[:])

            # W layout: per B-tile j a 32-col block; real 16 cols at offset
            # 16*(j%2): [8 wm block-diag | 8 pw block-diag].
            wdt = F32R if USE_F32R else F32
            # four independent 8-block W tiles so each psum group's matmuls
            # start as soon as its own slice is built (whole-tile deps)
            w2cs = []
            for gch in range(4):
                w2c = w_pool.tile([128, 8 * 32], wdt, tag="w2")
                w2cs.append(w2c)
                w2cf = w2c[:].bitcast(F32) if USE_F32R else w2c[:]
                nc.vector.memset(w2cf, 0.0)
                for srcT, off in ((wmT, 0), (pwT, 8)):
                    shifted = w2cf[:, off:]
                    out_view = bass.AP(
                        tensor=shifted.tensor, offset=shifted.offset,
                        ap=[list(shifted.ap[0]), [64, 4], [48, 2], [1, 8]],
                    )
                    nc.vector.tensor_tensor(
                        out_view,
                        srcT[:, 8 * gch:8 * (gch + 1)]
                            .rearrange("p (a b) -> p a b", a=4)
                            .unsqueeze(3).broadcast_to([128, 4, 2, 8]),
                        t_pat8[:].unsqueeze(1).unsqueeze(1)
                            .broadcast_to([128, 4, 2, 8]),
                        ALU.mult,
                    )

            # ---- Phase E: matmuls + ensemble CE per half-A-tile ----
            for hh in range(NH // NSG):
                hg = (NH // NSG) * sg + hh      # global half-A-tile 0..16
                pl = hh // 4                     # pair tile within sg
                xp = xps[pl]
                xpm = xp[:] if not USE_F32R else xp[:]
                psum = ps_mm.tile([128, 512], F32, tag="ps")
                for k in range(8):
                    tloc = 8 * (hh % 4) + k      # B-tile within pair
                    jl = 32 * pl + tloc          # W col-block within sg
                    v, s = k // 2, k % 2
                    nc.tensor.matmul(
                        psum[32 * v:32 * (v + 1), 0:C],
                        w2cs[hh][:, 32 * k:32 * (k + 1)],
                        xpm[:, C * tloc:C * (tloc + 1)],
                        start=(s == 0), stop=(s == 1),
                        tile_position=(0, 32 * v),
                    )
                # ensemble CE from the w rows (p rows computed too, unused)
                esl = st_pool.tile([128, 2], F32, tag="esl")
                eEW = st_pool.tile([128, C], F32, tag="eEW")
                nc.scalar.activation(eEW[:], psum[:, 0:C], ACTF.Exp,
                                     accum_out=esl[:, 1:2])
                li = idx_pool.tile([128, 1], I16, tag="li")
                nc.sync.dma_start(li[:], d_lidx[hg, :].unsqueeze(1))
                gth = st_pool.tile([128, 16], F32, tag="gth")
                nc.gpsimd.ap_gather(
                    gth[:].unsqueeze(2), eEW[:].unsqueeze(2), li[:],
                    channels=128, num_elems=C, d=1, num_idxs=16,
                )
                gscr = st_pool.tile([128, 16], F32, tag="gscr")
                nc.vector.tensor_mul(gscr[:], gth[:], t_diag[:])
                nc.vector.reduce_sum(esl[:, 0:1], gscr[:], axis=AX.X)
                lncols = st_pool.tile([128, 2], F32, tag="lncols")
                nc.scalar.activation(lncols[:], esl[:], ACTF.Ln)
                cet = st_pool.tile([128, 1], F32, tag="cet")
                nc.vector.tensor_sub(cet[:], lncols[:, 1:2], lncols[:, 0:1])
                nc.sync.dma_start(o_ce[hg, :].unsqueeze(1), cet[:])

                pstage = st_pool.tile([128, C], F32, tag="pstage")
                nc.scalar.copy(pstage[:], psum[:, 0:C])
                nc.sync.dma_start(o_ems[128 * hg:128 * (hg + 1), :],
                                  pstage[:])

    nc.compile()
    return nc


def _host_inputs(y_pred, labels):
    """Build the 8 per-core input maps."""
    x = np.ascontiguousarray(
        np.asarray(y_pred, dtype=np.float32).reshape(B, M * C))
    lab = np.asarray(labels).astype(np.int64).reshape(B)

    p = np.arange(128)
    pat8 = (np.arange(8)[None, :] == (p // 16)[:, None]).astype(np.float32)
    diag16 = (np.arange(16)[None, :] == (p % 16)[:, None]).astype(np.float32)
    ident = np.eye(128, dtype=np.float32)

    in_maps = []
    for cidx in range(NCORES):
        xl = x[BL * cidx:BL * (cidx + 1)].reshape(PAIRS, C)
        ll = lab[BL * cidx:BL * (cidx + 1)]
        gidx = np.zeros((NPAIR, 128, 2), np.int16)
        for pp in range(NPAIR):
            for w in range(2):
                i = (p % 16) + 16 * w
                q = p // 16
                gidx[pp, :, w] = (C * i + ll[256 * pp + 8 * i + q]).astype(
                    np.int16)
        lidx = np.zeros((NH, 128), np.int16)
        for hg in range(NH):
            a, hh = hg // 2, hg % 2
            lidx[hg] = ll[128 * a + 64 * hh + 8 * (p // 16) + (p % 8)].astype(
                np.int16)
        in_maps.append({
            "x": xl, "gidx": gidx, "lidx": lidx,
            "pat8": pat8, "diag16": diag16, "ident": ident,
        })
    return in_maps


def _host_finalize(results):
    ems = np.empty((B, C), np.float32)
    tcf = np.concatenate([r["tc_out"] for r in results], 0)
    xs = np.concatenate([r["xs_out"] for r in results], 0)
    ce = np.float64(0.0)
    ch = np.float64(0.0)
    cf = np.float64(0.0)
    for cidx, r in enumerate(results):
        # ems_raw rows: [hg (16), k (8), wp (2), b_off (8)] -> w rows wp=0
        raw = r["ems_raw"].reshape(NH, 8, 2, 8, C)
        # row b (within core) = 128a + 64hh + 8k + b_off ; hg = 2a+hh
        emsc = raw[:, :, 1, :, :]          # p rows  [NH, 8, 8, C]
        ems[BL * cidx:BL * (cidx + 1)] = emsc.reshape(BL, C)
        cep = r["ce_part"].reshape(NH, 8, 2, 8)
        ce += np.sum(cep[:, :, 0, :], dtype=np.float64)
        ch += np.sum(r["child_part"], dtype=np.float64)
        cf += np.sum(r["conf_part"], dtype=np.float64)
    child_loss = np.float32(-ch / (B * M))
    conf_loss = np.float32(cf / (B * M))
    ens_loss = np.float32(ce / B)
    return (ems, child_loss, conf_loss, ens_loss, xs, tcf)


_NC_CACHE = None


def kernel(y_pred, labels, class_type, model_num, num_local):
    assert int(class_type) == C and int(model_num) == M and int(num_local) == K
    global _NC_CACHE
    if _NC_CACHE is None:
        _NC_CACHE = build_nc()
    nc = _NC_CACHE
    in_maps = _host_inputs(y_pred, labels)

    from concourse.bass_utils import run_bass_kernel_spmd
    res = run_bass_kernel_spmd(
        nc, in_maps, list(range(NCORES)),
        trace=bool(int(os.environ.get("KERNEL_TRACE", "0"))),
    )
    out = _host_finalize(res.results)
    if res.exec_time_ns is not None:
        kernel.last_exec_time_ns = res.exec_time_ns
        kernel.last_mean_exec_time_ns = res.mean_exec_time_ns
    return out
